# revision 17
# baseline (speedup 1.0000x reference)
"""BiLSTM-CRF loss kernel for Trainium2 (8 NeuronCores, data-parallel over batch).

Strategy:
  - Batch (128) split 8 ways -> 16 sequences per core. No collectives; host
    sums the per-sequence log-likelihoods and takes -mean.
  - Per core:
    Phase A: embedding gather (indirect DMA) + bulk input-projection GEMM
             (x @ [Wih_f;Wih_b]^T + biases) -> xproj in DRAM. f32r matmuls.
    Phase B: fused fw+bw LSTM recurrence, both directions in-phase per step
             (shared ACT calls on stacked [32, *] tiles). Transposed hidden
             states (lhsT layout) for every step are kept in SBUF.
    Phase B': bulk output projection h -> emissions (bf16 in SBUF).
    Phase C: two-sided CRF: forward alpha (t=1..127) and backward beta
             (t=255..128) recursions run together; the T x T logsumexp is
             computed as exp/matmul/log with exp(trans + bout) folded in.
             Gold-path emission scores accumulated with iota/is_equal masks.
  - Host precomputes only index/permutation transforms of the integer inputs
    (gate reorder of weights, tag/length masks, static numerator terms).
"""

import os
import numpy as np

V, E, H, T, B, L = 50000, 512, 256, 74, 128, 256
G = 4 * H            # 1024 gates per direction
NC = 8
BC = B // NC         # 16 sequences per core
R = L * BC           # 4096 rows (t-major) per core
KMID = L // 2 - 1    # 127: alpha covers t=1..127, beta covers t=255..128

_CACHE = {}


def _build():
    from contextlib import ExitStack

    import concourse.bass as bass
    import concourse.mybir as mybir
    import concourse.tile as tile
    from concourse import bacc
    from concourse.masks import make_identity

    F32 = mybir.dt.float32
    F32R = mybir.dt.float32r
    BF16 = mybir.dt.bfloat16
    I32 = mybir.dt.int32
    AF = mybir.ActivationFunctionType
    ALU = mybir.AluOpType
    AX = mybir.AxisListType

    nc = bacc.Bacc("TRN2", target_bir_lowering=False, debug=False, num_devices=NC)

    def din(name, shape, dt=F32):
        return nc.dram_tensor(name, shape, dt, kind="ExternalInput").ap()

    embed_d = din("embed", [V, E])
    ids_d = din("ids", [128, R // 128], I32)
    wallT_d = din("wallT", [4, 128, 2 * G])
    bias_d = din("bias_all", [128, 2 * G])
    whhT_d = din("whhT", [2, 2, 128, G])
    woutT_d = din("woutT", [2, 2, 128, T])
    h0_d = din("h0c", [2 * BC, H])
    c0_d = din("c0c", [2 * BC, H])
    trans_d = din("transm", [T, T])
    bout_d = din("bout74", [T, T])
    bout16_d = din("bout16", [BC, T])
    start_d = din("start16", [BC, T])
    end_d = din("end16", [BC, T])
    iota_d = din("iota16", [BC, T])
    tags_d = din("tagsf", [BC, L])
    maskv_d = din("maskv", [BC, L], I32)
    nstat_d = din("nstat", [BC, 1])
    injb_d = din("injb", [48, 32])
    llh_d = nc.dram_tensor("llh", [BC, 1], F32, kind="ExternalOutput").ap()

    NCH = R // 128  # 32 gather/GEMM chunks

    with tile.TileContext(nc) as tc, ExitStack() as ctx:
        dram = ctx.enter_context(tc.tile_pool(name="dram", bufs=1, space="DRAM"))
        xproj = dram.tile([R, 2 * G], F32)

        persist = ctx.enter_context(tc.tile_pool(name="persist", bufs=1))
        ident = persist.tile([128, 128], F32)
        make_identity(nc, ident[:])

        # ---- persistent state ----
        hT_all = persist.tile([128, 64 * (L + 1)], F32R)
        em_f = dram.tile([L * BC, T], F32)
        em_b = dram.tile([L * BC, T], F32)
        tags_sb = persist.tile([BC, L], F32)
        nc.sync.dma_start(tags_sb[:], tags_d[:])
        maskv_sb = persist.tile([BC, L], I32)
        nc.sync.dma_start(maskv_sb[:], maskv_d[:])

        # ================= Phase A: gather + input projection =================
        with ExitStack() as actx:
            wa = actx.enter_context(tc.tile_pool(name="wa", bufs=1))
            ga = actx.enter_context(tc.tile_pool(name="ga", bufs=3))
            xt = actx.enter_context(tc.tile_pool(name="xt", bufs=3))
            tpp = actx.enter_context(tc.tile_pool(name="tpp", bufs=3, space="PSUM"))
            gep = actx.enter_context(tc.tile_pool(name="gep", bufs=3, space="PSUM"))
            oa = actx.enter_context(tc.tile_pool(name="oa", bufs=4))

            ids_sb = wa.tile([128, NCH], I32)
            nc.sync.dma_start(ids_sb[:], ids_d[:])
            wallT_t = []
            for q in range(4):
                wt = wa.tile([128, 2 * G], F32R, name=f"wallT{q}")
                nc.gpsimd.dma_start(wt[:], wallT_d[q])
                wallT_t.append(wt)
            bias_sb = wa.tile([128, 2 * G], F32)
            nc.sync.dma_start(bias_sb[:], bias_d[:])

            for g in range(NCH):
                xg = ga.tile([128, E], F32, tag="xg")
                nc.gpsimd.indirect_dma_start(
                    out=xg[:], out_offset=None, in_=embed_d,
                    in_offset=bass.IndirectOffsetOnAxis(ap=ids_sb[:, g:g + 1], axis=0))
                xTt = xt.tile([128, E], F32R, tag="xT")
                for q in range(4):
                    tp = tpp.tile([128, 128], F32, tag="tpA")
                    nc.tensor.transpose(tp[:], xg[:, 128 * q:128 * (q + 1)], ident[:])
                    nc.vector.tensor_copy(xTt[:, 128 * q:128 * (q + 1)], tp[:])
                for nh in range(4):
                    pso = gep.tile([128, 512], F32, tag="gemm")
                    for q in range(4):
                        nc.tensor.matmul(
                            pso[:], xTt[:, 128 * q:128 * (q + 1)],
                            wallT_t[q][:, 512 * nh:512 * (nh + 1)],
                            start=(q == 0), stop=(q == 3))
                    ob = oa.tile([128, 512], F32, tag="ob")
                    nc.vector.tensor_tensor(
                        out=ob[:], in0=pso[:],
                        in1=bias_sb[:, 512 * nh:512 * (nh + 1)],
                        op=ALU.add)
                    nc.sync.dma_start(
                        xproj[128 * g:128 * (g + 1), 512 * nh:512 * (nh + 1)], ob[:])

        # ================= Phase B: fused BiLSTM recurrence =================
        with ExitStack() as bctx:
            wb = bctx.enter_context(tc.tile_pool(name="wb", bufs=1))
            xpp = bctx.enter_context(tc.tile_pool(name="xpp", bufs=4))
            gps = bctx.enter_context(tc.tile_pool(name="gps", bufs=1, space="PSUM"))
            tps = bctx.enter_context(tc.tile_pool(name="tps", bufs=2, space="PSUM"))
            sgp = bctx.enter_context(tc.tile_pool(name="sgp", bufs=2))
            cvp = bctx.enter_context(tc.tile_pool(name="cvp", bufs=2))
            tmp = bctx.enter_context(tc.tile_pool(name="tmp", bufs=4))

            whhT_t = [[None, None], [None, None]]
            for d in range(2):
                for kp in range(2):
                    wt = wb.tile([128, G], F32R, name=f"whhT{d}{kp}")
                    nc.gpsimd.dma_start(wt[:], whhT_d[d, kp])
                    whhT_t[d][kp] = wt
            i48 = wb.tile([48, 48], F32R)
            nc.vector.tensor_copy(i48[:], ident[:48, :48])

            h0sb = wb.tile([48, H], F32)
            nc.sync.dma_start(h0sb[0:16, :], h0_d[0:BC])
            nc.sync.dma_start(h0sb[32:48, :], h0_d[BC:2 * BC])
            c_cur = [None, None]
            for d in range(2):
                ct = cvp.tile([BC, H], F32, tag=f"c{d}")
                nc.sync.dma_start(ct[:], c0_d[BC * d:BC * (d + 1)])
                c_cur[d] = ct
            # initial hT: slot layout per 64 cols: fw_k0 | bw_k0 | fw_k1 | bw_k1
            for d in range(2):
                ro = 32 * d
                for half in range(2):
                    tp = tps.tile([128, 16], F32, tag="tpB")
                    nc.tensor.transpose(tp[:], h0sb[ro:ro + 16,
                                        128 * half:128 * (half + 1)],
                                        ident[ro:ro + 16, ro:ro + 16],
                                        tile_position=(ro, 0))
                    nc.vector.tensor_copy(
                        hT_all[:, 32 * half + 16 * d:32 * half + 16 * d + 16], tp[:])

            for t in range(L):
                base = 64 * t
                nbase = 64 * (t + 1)
                xp = xpp.tile([48, G], F32R, tag="xp")
                nc.gpsimd.dma_start(xp[0:BC, :], xproj[BC * t:BC * (t + 1), 0:G])
                tb_ = L - 1 - t
                nc.gpsimd.dma_start(xp[32:48, :], xproj[BC * tb_:BC * (tb_ + 1), G:2 * G])

                for d in range(2):
                    ro = 32 * d
                    gt = gps.tile([BC, G], F32, tag=f"g{d}")
                    idd = i48[ro:ro + 16, ro:ro + 16]
                    for nh in range(2):
                        nc.tensor.matmul(gt[:, 512 * nh:512 * (nh + 1)], idd,
                                         xp[ro:ro + 16, 512 * nh:512 * (nh + 1)],
                                         start=True, stop=False,
                                         skip_group_check=True,
                                         tile_position=(ro, 0))
                        for kp in range(2):
                            lhsT = hT_all[:, base + 32 * kp + 16 * d:
                                          base + 32 * kp + 16 * d + 16]
                            nc.tensor.matmul(gt[:, 512 * nh:512 * (nh + 1)], lhsT,
                                             whhT_t[d][kp][:, 512 * nh:512 * (nh + 1)],
                                             start=False, stop=(kp == 1),
                                             skip_group_check=True)
                    sg = sgp.tile([BC, G], F32, tag=f"sg{d}")
                    nc.scalar.activation(sg[:, 0:512], gt[:, 0:512], AF.Sigmoid)
                    nc.scalar.activation(sg[:, 512:768], gt[:, 512:768], AF.Sigmoid)
                    nc.scalar.activation(sg[:, 768:1024], gt[:, 768:1024], AF.Tanh)

                    t1 = tmp.tile([BC, H], F32, tag=f"t1{d}")
                    nc.vector.tensor_tensor(out=t1[:], in0=sg[:, 256:512],
                                            in1=c_cur[d][:], op=ALU.mult)
                    t2 = tmp.tile([BC, H], F32, tag=f"t2{d}")
                    nc.gpsimd.tensor_tensor(out=t2[:], in0=sg[:, 0:256],
                                            in1=sg[:, 768:1024], op=ALU.mult)
                    c_new = cvp.tile([BC, H], F32, tag=f"c{d}")
                    nc.vector.tensor_tensor(out=c_new[:], in0=t1[:], in1=t2[:],
                                            op=ALU.add)
                    th = tmp.tile([BC, H], F32, tag=f"th{d}")
                    nc.scalar.activation(th[:], c_new[:], AF.Tanh)
                    hnew = tmp.tile([BC, H], F32, tag=f"h{d}")
                    nc.vector.tensor_tensor(out=hnew[:], in0=sg[:, 512:768],
                                            in1=th[:], op=ALU.mult)
                    for half in range(2):
                        tp = tps.tile([128, 16], F32, tag="tpB")
                        nc.tensor.transpose(tp[:], hnew[:, 128 * half:128 * (half + 1)],
                                            ident[:16, :16])
                        nc.vector.tensor_copy(
                            hT_all[:, nbase + 32 * half + 16 * d:
                                   nbase + 32 * half + 16 * d + 16], tp[:])
                    c_cur[d] = c_new

            # ---- Phase B': bulk output projection -> emissions ----
            woutT_t = [[None, None], [None, None]]
            for d in range(2):
                for kp in range(2):
                    wt = wb.tile([128, T], F32R, name=f"woutT{d}{kp}")
                    nc.gpsimd.dma_start(wt[:], woutT_d[d, kp])
                    woutT_t[d][kp] = wt

            psE = bctx.enter_context(tc.tile_pool(name="psE", bufs=2, space="PSUM"))
            for d in range(2):
                for j in range(NCH):
                    pse = psE.tile([128, T], F32, tag="pse")
                    s0 = 64 * (8 * j + 1)
                    blk = hT_all[:, s0:s0 + 512].rearrange("p (t u) -> p t u", u=64)
                    for kp in range(2):
                        a0 = 32 * kp + 16 * d
                        bstg = tmp.tile([128, 128], F32R, tag="bstg")
                        nc.vector.tensor_copy(bstg[:], blk[:, :, a0:a0 + 16])
                        nc.tensor.matmul(pse[:], bstg[:], woutT_t[d][kp],
                                         start=(kp == 0), stop=(kp == 1))
                    stg = tmp.tile([128, T], F32, tag="stg")
                    nc.vector.tensor_copy(stg[:], pse[:])
                    dst = em_f if d == 0 else em_b
                    for ti in range(8):
                        sx = 8 * j + ti
                        pos = sx if d == 0 else L - 1 - sx
                        nc.sync.dma_start(dst[BC * pos:BC * (pos + 1), :],
                                          stg[16 * ti:16 * (ti + 1), :])

        # ================= Phase C: two-sided CRF =================
        with ExitStack() as cctx:
            wc = cctx.enter_context(tc.tile_pool(name="wc", bufs=1))
            cps = cctx.enter_context(tc.tile_pool(name="cps", bufs=2, space="PSUM"))
            qps = cctx.enter_context(tc.tile_pool(name="qps", bufs=2, space="PSUM"))
            ctm = cctx.enter_context(tc.tile_pool(name="ctm", bufs=6))

            trs = wc.tile([T, T], F32)
            nc.sync.dma_start(trs[:], trans_d[:])
            bout74_sb = wc.tile([T, T], F32)
            nc.sync.dma_start(bout74_sb[:], bout_d[:])
            bout16_sb = wc.tile([BC, T], F32)
            nc.sync.dma_start(bout16_sb[:], bout16_d[:])
            start_sb = wc.tile([BC, T], F32)
            nc.sync.dma_start(start_sb[:], start_d[:])
            end_sb = wc.tile([BC, T], F32)
            nc.sync.dma_start(end_sb[:], end_d[:])
            nstat_sb = wc.tile([BC, 1], F32)
            nc.sync.dma_start(nstat_sb[:], nstat_d[:])

            trb = wc.tile([T, T], F32)
            nc.vector.tensor_tensor(out=trb[:], in0=trs[:], in1=bout74_sb[:],
                                    op=ALU.add)
            ET = wc.tile([T, T], F32)
            nc.scalar.activation(ET[:], trb[:], AF.Exp)
            etp = cps.tile([T, T], F32, tag="etp")
            nc.tensor.transpose(etp[:], ET[:], ident[:T, :T])
            ETT = wc.tile([T, T], F32)
            nc.vector.tensor_copy(ETT[:], etp[:])

            sv = wc.tile([BC, T], F32)
            nc.vector.tensor_tensor(out=sv[:], in0=start_sb[:], in1=bout16_sb[:],
                                    op=ALU.add)
            iota_f = wc.tile([BC, T], F32)
            nc.sync.dma_start(iota_f[:], iota_d[:])

            alpha = wc.tile([BC, T], F32)
            beta = wc.tile([BC, T], F32)
            acc_a = wc.tile([BC, T], F32)
            acc_b = wc.tile([BC, T], F32)

            def load_em(t, tagp):
                ef = ctm.tile([BC, T], F32, tag=tagp + "f")
                nc.sync.dma_start(ef[:], em_f[BC * t:BC * (t + 1), :])
                eb = ctm.tile([BC, T], F32, tag=tagp + "b")
                nc.sync.dma_start(eb[:], em_b[BC * t:BC * (t + 1), :])
                es = ctm.tile([BC, T], F32, tag=tagp + "s")
                nc.vector.tensor_tensor(out=es[:], in0=ef[:], in1=eb[:], op=ALU.add)
                return es

            ema0 = load_em(0, "e0")
            nc.vector.tensor_tensor(out=alpha[:], in0=ema0[:],
                                    in1=sv[:], op=ALU.add)
            eq0 = ctm.tile([BC, T], F32, tag="eq")
            nc.vector.tensor_scalar(out=eq0[:], in0=iota_f[:],
                                    scalar1=tags_sb[:, 0:1], scalar2=None, op0=ALU.is_equal)
            nc.vector.tensor_tensor(out=acc_a[:], in0=eq0[:], in1=ema0[:], op=ALU.mult)
            nc.vector.tensor_copy(beta[:], end_sb[:])
            nc.gpsimd.memset(acc_b[:], 0.0)

            def crf_step(k):
                do_a = k <= KMID
                ta, tb2 = k, L - k
                pt = ctm.tile([48, T], F32, tag="pt")
                if do_a:
                    ema = load_em(ta, "ea")
                    nm = ctm.tile([BC, 1], F32, tag="nm")
                    nc.vector.tensor_reduce(nm[:], alpha[:], axis=AX.X, op=ALU.max,
                                            negate=True)
                    nc.scalar.activation(pt[:BC, :], alpha[:], AF.Exp, bias=nm[:, :1])
                emb2 = load_em(tb2, "eb")
                ub = ctm.tile([BC, T], F32, tag="ub")
                nc.vector.tensor_tensor(out=ub[:], in0=emb2[:], in1=beta[:], op=ALU.add)
                nmb = ctm.tile([BC, 1], F32, tag="nmb")
                nc.vector.tensor_reduce(nmb[:], ub[:], axis=AX.X, op=ALU.max,
                                        negate=True)
                nc.scalar.activation(pt[32:48, :], ub[:], AF.Exp, bias=nmb[:, :1])

                ptp = cps.tile([T, 48], F32, tag="ptp")
                nc.tensor.transpose(ptp[:], pt[:], ident[:48, :48])
                pts = ctm.tile([T, 48], F32, tag="pts")
                nc.vector.tensor_copy(pts[:], ptp[:])

                if do_a:
                    qa = qps.tile([BC, T], F32, tag="qa")
                    nc.tensor.matmul(qa[:], pts[:, 0:BC], ET[:], start=True, stop=True)
                    la = ctm.tile([BC, T], F32, tag="la")
                    nc.scalar.activation(la[:], qa[:], AF.Ln)
                    nxa = ctm.tile([BC, T], F32, tag="nxa")
                    nc.vector.scalar_tensor_tensor(out=nxa[:], in0=la[:],
                                                   scalar=nm[:, :1], in1=ema[:],
                                                   op0=ALU.subtract, op1=ALU.add)
                    nc.vector.copy_predicated(
                        out=alpha[:], mask=maskv_sb[:, ta:ta + 1].to_broadcast([BC, T]),
                        data=nxa[:])
                    eqa = ctm.tile([BC, T], F32, tag="eq")
                    nc.vector.tensor_scalar(out=eqa[:],
                                            in0=iota_f[:],
                                            scalar1=tags_sb[:, ta:ta + 1],
                                            scalar2=None, op0=ALU.is_equal)
                    ga = ctm.tile([BC, T], F32, tag="ga")
                    nc.vector.tensor_tensor(out=ga[:], in0=eqa[:], in1=ema[:],
                                            op=ALU.mult)
                    nc.vector.tensor_tensor(out=acc_a[:], in0=acc_a[:], in1=ga[:],
                                            op=ALU.add)

                qb = qps.tile([BC, T], F32, tag="qb")
                nc.tensor.matmul(qb[:], pts[:, 32:48], ETT[:], start=True, stop=True)
                lb = ctm.tile([BC, T], F32, tag="lb")
                nc.scalar.activation(lb[:], qb[:], AF.Ln)
                nxb = ctm.tile([BC, T], F32, tag="nxb")
                nc.vector.tensor_scalar(out=nxb[:], in0=lb[:], scalar1=nmb[:, :1],
                                        scalar2=None, op0=ALU.subtract)
                nc.vector.copy_predicated(
                    out=beta[:], mask=maskv_sb[:, tb2:tb2 + 1].to_broadcast([BC, T]),
                    data=nxb[:])
                eqb = ctm.tile([BC, T], F32, tag="eq")
                nc.vector.tensor_scalar(out=eqb[:],
                                        in0=iota_f[:],
                                        scalar1=tags_sb[:, tb2:tb2 + 1],
                                        scalar2=None, op0=ALU.is_equal)
                gb = ctm.tile([BC, T], F32, tag="gb")
                nc.vector.tensor_tensor(out=gb[:], in0=eqb[:], in1=emb2[:], op=ALU.mult)
                nc.vector.tensor_tensor(out=acc_b[:], in0=acc_b[:], in1=gb[:],
                                        op=ALU.add)

            for k in range(1, L - KMID):
                crf_step(k)

            # ---- finalize: den = lse(alpha + beta), llh = nstat + gold - den ----
            uf = ctm.tile([BC, T], F32, tag="uf")
            nc.vector.tensor_tensor(out=uf[:], in0=alpha[:], in1=beta[:], op=ALU.add)
            nmf = wc.tile([BC, 1], F32)
            nc.vector.tensor_reduce(nmf[:], uf[:], axis=AX.X, op=ALU.max, negate=True)
            pf = ctm.tile([BC, T], F32, tag="pf")
            sf = wc.tile([BC, 1], F32)
            nc.scalar.activation(pf[:], uf[:], AF.Exp, bias=nmf[:, :1],
                                 accum_out=sf[:, :1])
            lnf = wc.tile([BC, 1], F32)
            nc.scalar.activation(lnf[:], sf[:], AF.Ln)
            den = wc.tile([BC, 1], F32)
            nc.vector.tensor_tensor(out=den[:], in0=lnf[:], in1=nmf[:], op=ALU.subtract)
            gsa = wc.tile([BC, 1], F32)
            nc.vector.tensor_reduce(gsa[:], acc_a[:], axis=AX.X, op=ALU.add)
            gsb = wc.tile([BC, 1], F32)
            nc.vector.tensor_reduce(gsb[:], acc_b[:], axis=AX.X, op=ALU.add)
            r1 = wc.tile([BC, 1], F32)
            nc.vector.tensor_tensor(out=r1[:], in0=gsa[:], in1=gsb[:], op=ALU.add)
            r2 = wc.tile([BC, 1], F32)
            nc.vector.tensor_tensor(out=r2[:], in0=r1[:], in1=nstat_sb[:], op=ALU.add)
            llh_t = wc.tile([BC, 1], F32)
            nc.vector.tensor_tensor(out=llh_t[:], in0=r2[:], in1=den[:], op=ALU.subtract)
            nc.sync.dma_start(llh_d[:], llh_t[:])

    nc.compile()
    return nc


def _injb_const():
    m = np.zeros((48, 32), np.float32)
    m[32:48, 0:16] = np.eye(16, dtype=np.float32)
    return m


def _host_prep(inputs):
    """Pure index/permutation transforms of the inputs -> per-core input maps."""
    f32 = np.float32
    ii = {k: np.asarray(v) for k, v in inputs.items()}
    input_ids = ii["input_ids"].astype(np.int32)
    tag_ids = ii["tag_ids"].astype(np.int64)
    lengths = np.maximum(ii["lengths"].astype(np.int64), 1)
    embed = np.ascontiguousarray(ii["embed"], dtype=f32)
    trans = np.asarray(ii["trans"], f32)
    start_t = np.asarray(ii["start_t"], f32)
    end_t = np.asarray(ii["end_t"], f32)
    bout = np.asarray(ii["bout"], f32)

    # gate reorder: torch [i, f, g, o] -> [i, f, o, g]
    perm = np.concatenate([np.arange(0, 256), np.arange(256, 512),
                           np.arange(768, 1024), np.arange(512, 768)])
    Wih_f = np.asarray(ii["Wih_f"], f32)[perm]
    Wih_b = np.asarray(ii["Wih_b"], f32)[perm]
    Whh_f = np.asarray(ii["Whh_f"], f32)[perm]
    Whh_b = np.asarray(ii["Whh_b"], f32)[perm]
    b_f = (np.asarray(ii["bih_f"], f32) + np.asarray(ii["bhh_f"], f32))[perm]
    b_b = (np.asarray(ii["bih_b"], f32) + np.asarray(ii["bhh_b"], f32))[perm]
    Wout = np.asarray(ii["Wout"], f32)

    W_all = np.concatenate([Wih_f, Wih_b], axis=0)          # [2G, E]
    wallT = np.ascontiguousarray(W_all.T.reshape(4, 128, 2 * G))
    bias_all = np.concatenate([b_f, b_b])[None, :]          # [1, 2G]
    whhT = np.stack([np.ascontiguousarray(Whh_f.T.reshape(2, 128, G)),
                     np.ascontiguousarray(Whh_b.T.reshape(2, 128, G))])
    woutT = np.stack([np.ascontiguousarray(Wout[:, :H].T.reshape(2, 128, T)),
                      np.ascontiguousarray(Wout[:, H:].T.reshape(2, 128, T))])
    h0 = np.asarray(ii["h0"], f32)
    c0 = np.asarray(ii["c0"], f32)

    shared = dict(embed=embed, wallT=wallT,
                  bias_all=np.ascontiguousarray(np.broadcast_to(bias_all, (128, 2 * G))),
                  whhT=whhT, woutT=woutT, transm=trans,
                  bout74=np.ascontiguousarray(np.broadcast_to(bout[None, :], (T, T))),
                  bout16=np.ascontiguousarray(np.broadcast_to(bout[None, :], (BC, T))),
                  start16=np.ascontiguousarray(np.broadcast_to(start_t[None, :], (BC, T))),
                  end16=np.ascontiguousarray(np.broadcast_to(end_t[None, :], (BC, T))),
                  iota16=np.ascontiguousarray(np.broadcast_to(
                      np.arange(T, dtype=f32)[None, :], (BC, T))),
                  injb=_injb_const())

    tpos = np.arange(L)
    in_maps = []
    for c in range(NC):
        sl = slice(c * BC, (c + 1) * BC)
        ids_lin = input_ids[sl].T.reshape(-1)               # t-major [R]
        ids_arr = np.ascontiguousarray(ids_lin.reshape(R // 128, 128).T).astype(np.int32)
        tg = tag_ids[sl]                                    # [BC, L]
        ln = lengths[sl]                                    # [BC]
        valid = tpos[None, :] < ln[:, None]                 # [BC, L]
        tags_eff = np.where(valid, tg, 999).astype(f32)
        maskv = valid.astype(np.int32)
        # static numerator terms (start/trans/end/bout lookups)
        nstat = start_t[tg[:, 0]].astype(np.float64)
        m = valid[:, 1:]
        nstat += (trans[tg[:, :-1], tg[:, 1:]].astype(np.float64) * m).sum(axis=1)
        nstat += end_t[tg[np.arange(BC), ln - 1]]
        nstat += (bout[tg].astype(np.float64) * valid).sum(axis=1)
        im = dict(shared)
        im.update(ids=ids_arr, tagsf=np.ascontiguousarray(tags_eff),
                  maskv=np.ascontiguousarray(maskv),
                  nstat=nstat.astype(f32)[:, None],
                  h0c=np.ascontiguousarray(h0[:, sl].reshape(2 * BC, H)),
                  c0c=np.ascontiguousarray(c0[:, sl].reshape(2 * BC, H)))
        in_maps.append(im)
    return in_maps


def kernel(**inputs):
    from concourse.bass_utils import run_bass_kernel_spmd

    if "nc" not in _CACHE:
        _CACHE["nc"] = _build()
    nc = _CACHE["nc"]
    in_maps = _host_prep(inputs)
    trace = bool(int(os.environ.get("KERNEL_TRACE", "0")))
    res = run_bass_kernel_spmd(nc, in_maps, core_ids=list(range(NC)), trace=trace)
    if trace:
        _CACHE["exec_time_ns"] = res.exec_time_ns
    llh = np.concatenate([r["llh"][:, 0] for r in res.results])
    return np.asarray(-llh.mean(), dtype=np.float32)


# revision 22
# speedup vs baseline: 14.7095x; 14.7095x over previous
"""BiLSTM-CRF loss kernel for Trainium2 (8 NeuronCores, data-parallel over batch).

Strategy:
  - Batch (128) split 8 ways -> 16 sequences per core. No collectives; host
    sums the per-sequence log-likelihoods and takes -mean.
  - Per core:
    Phase A: embedding gather (indirect DMA) + bulk input-projection GEMM
             (x @ [Wih_f;Wih_b]^T + biases) -> xproj in DRAM. f32r matmuls.
    Phase B: fused fw+bw LSTM recurrence, both directions in-phase per step
             (shared ACT calls on stacked [32, *] tiles). Transposed hidden
             states (lhsT layout) for every step are kept in SBUF.
    Phase B': bulk output projection h -> emissions (bf16 in SBUF).
    Phase C: two-sided CRF: forward alpha (t=1..127) and backward beta
             (t=255..128) recursions run together; the T x T logsumexp is
             computed as exp/matmul/log with exp(trans + bout) folded in.
             Gold-path emission scores accumulated with iota/is_equal masks.
  - Host precomputes only index/permutation transforms of the integer inputs
    (gate reorder of weights, tag/length masks, static numerator terms).
"""

import os
import numpy as np

V, E, H, T, B, L = 50000, 512, 256, 74, 128, 256
G = 4 * H            # 1024 gates per direction
NC = 8
BC = B // NC         # 16 sequences per core
R = L * BC           # 4096 rows (t-major) per core
KMID = L // 2 - 1    # 127: alpha covers t=1..127, beta covers t=255..128

_CACHE = {}


def _build():
    from contextlib import ExitStack

    import concourse.bass as bass
    import concourse.mybir as mybir
    import concourse.tile as tile
    from concourse import bacc
    from concourse.masks import make_identity

    F32 = mybir.dt.float32
    F32R = mybir.dt.float32r
    BF16 = mybir.dt.bfloat16
    I32 = mybir.dt.int32
    AF = mybir.ActivationFunctionType
    ALU = mybir.AluOpType
    AX = mybir.AxisListType

    nc = bacc.Bacc("TRN2", target_bir_lowering=False, debug=False, num_devices=NC)

    def din(name, shape, dt=F32):
        return nc.dram_tensor(name, shape, dt, kind="ExternalInput").ap()

    embed_d = din("embed", [V, E])
    ids_d = din("ids", [128, R // 128], I32)
    wallT_d = din("wallT", [4, 128, 2 * G])
    bias_d = din("bias_all", [128, 2 * G])
    whhT_d = din("whhT", [2, 2, 128, G])
    woutT_d = din("woutT", [2, 2, 128, T])
    h0_d = din("h0c", [2 * BC, H])
    c0_d = din("c0c", [2 * BC, H])
    trans_d = din("transm", [T, T])
    bout_d = din("bout74", [T, T])
    bout16_d = din("bout16", [BC, T])
    start_d = din("start16", [BC, T])
    end_d = din("end16", [BC, T])
    iota_d = din("iota16", [BC, T])
    tags_d = din("tagsf", [BC, L])
    maskv_d = din("maskv", [BC, L], I32)
    nstat_d = din("nstat", [BC, 1])
    injb_d = din("injb", [48, 32])
    llh_d = nc.dram_tensor("llh", [BC, 1], F32, kind="ExternalOutput").ap()

    NCH = R // 128  # 32 gather/GEMM chunks

    with tile.TileContext(nc) as tc, ExitStack() as ctx:
        dram = ctx.enter_context(tc.tile_pool(name="dram", bufs=1, space="DRAM"))
        xproj = dram.tile([R, 2 * G], F32)

        persist = ctx.enter_context(tc.tile_pool(name="persist", bufs=1))
        ident = persist.tile([128, 128], F32)
        make_identity(nc, ident[:])

        # ---- persistent state ----
        hT_all = persist.tile([128, 64 * (L + 1)], F32R)
        em_f = dram.tile([L * BC, T], F32)
        em_b = dram.tile([L * BC, T], F32)
        tags_sb = persist.tile([BC, L], F32)
        nc.sync.dma_start(tags_sb[:], tags_d[:])
        maskv_sb = persist.tile([BC, L], I32)
        nc.sync.dma_start(maskv_sb[:], maskv_d[:])

        # ================= Phase A: gather + input projection =================
        with ExitStack() as actx:
            wa = actx.enter_context(tc.tile_pool(name="wa", bufs=1))
            ga = actx.enter_context(tc.tile_pool(name="ga", bufs=3))
            xt = actx.enter_context(tc.tile_pool(name="xt", bufs=3))
            tpp = actx.enter_context(tc.tile_pool(name="tpp", bufs=3, space="PSUM"))
            gep = actx.enter_context(tc.tile_pool(name="gep", bufs=3, space="PSUM"))
            oa = actx.enter_context(tc.tile_pool(name="oa", bufs=4))

            ids_sb = wa.tile([128, NCH], I32)
            nc.sync.dma_start(ids_sb[:], ids_d[:])
            wallT_t = []
            for q in range(4):
                wt = wa.tile([128, 2 * G], F32R, name=f"wallT{q}")
                nc.gpsimd.dma_start(wt[:], wallT_d[q])
                wallT_t.append(wt)
            bias_sb = wa.tile([128, 2 * G], F32)
            nc.sync.dma_start(bias_sb[:], bias_d[:])

            for g in range(NCH):
                xg = ga.tile([128, E], F32, tag="xg")
                nc.gpsimd.indirect_dma_start(
                    out=xg[:], out_offset=None, in_=embed_d,
                    in_offset=bass.IndirectOffsetOnAxis(ap=ids_sb[:, g:g + 1], axis=0))
                xTt = xt.tile([128, E], F32R, tag="xT")
                for q in range(4):
                    tp = tpp.tile([128, 128], F32, tag="tpA")
                    nc.tensor.transpose(tp[:], xg[:, 128 * q:128 * (q + 1)], ident[:])
                    nc.vector.tensor_copy(xTt[:, 128 * q:128 * (q + 1)], tp[:])
                for nh in range(4):
                    pso = gep.tile([128, 512], F32, tag="gemm")
                    for q in range(4):
                        nc.tensor.matmul(
                            pso[:], xTt[:, 128 * q:128 * (q + 1)],
                            wallT_t[q][:, 512 * nh:512 * (nh + 1)],
                            start=(q == 0), stop=(q == 3))
                    ob = oa.tile([128, 512], F32, tag="ob")
                    nc.vector.tensor_tensor(
                        out=ob[:], in0=pso[:],
                        in1=bias_sb[:, 512 * nh:512 * (nh + 1)],
                        op=ALU.add)
                    nc.sync.dma_start(
                        xproj[128 * g:128 * (g + 1), 512 * nh:512 * (nh + 1)], ob[:])

        # ================= Phase B: fused BiLSTM recurrence =================
        with ExitStack() as bctx:
            wb = bctx.enter_context(tc.tile_pool(name="wb", bufs=1))
            xpp = bctx.enter_context(tc.tile_pool(name="xpp", bufs=4))
            gps = bctx.enter_context(tc.tile_pool(name="gps", bufs=1, space="PSUM"))
            tps = bctx.enter_context(tc.tile_pool(name="tps", bufs=2, space="PSUM"))
            sgp = bctx.enter_context(tc.tile_pool(name="sgp", bufs=2))
            cvp = bctx.enter_context(tc.tile_pool(name="cvp", bufs=2))
            tmp = bctx.enter_context(tc.tile_pool(name="tmp", bufs=4))

            whhT_t = [[None, None], [None, None]]
            for d in range(2):
                for kp in range(2):
                    wt = wb.tile([128, G], F32R, name=f"whhT{d}{kp}")
                    nc.gpsimd.dma_start(wt[:], whhT_d[d, kp])
                    whhT_t[d][kp] = wt
            i48 = wb.tile([48, 48], F32R)
            nc.vector.tensor_copy(i48[:], ident[:48, :48])

            h0sb = wb.tile([48, H], F32)
            nc.sync.dma_start(h0sb[0:16, :], h0_d[0:BC])
            nc.sync.dma_start(h0sb[32:48, :], h0_d[BC:2 * BC])
            c_cur = [None, None]
            for d in range(2):
                ct = cvp.tile([BC, H], F32, tag=f"c{d}")
                nc.sync.dma_start(ct[:], c0_d[BC * d:BC * (d + 1)])
                c_cur[d] = ct
            # initial hT: slot layout per 64 cols: fw_k0 | bw_k0 | fw_k1 | bw_k1
            for d in range(2):
                ro = 32 * d
                for half in range(2):
                    tp = tps.tile([128, 16], F32, tag="tpB")
                    nc.tensor.transpose(tp[:], h0sb[ro:ro + 16,
                                        128 * half:128 * (half + 1)],
                                        ident[ro:ro + 16, ro:ro + 16],
                                        tile_position=(ro, 0))
                    nc.vector.tensor_copy(
                        hT_all[:, 32 * half + 16 * d:32 * half + 16 * d + 16], tp[:])

            for t in range(L):
                base = 64 * t
                nbase = 64 * (t + 1)
                xp = xpp.tile([48, G], F32R, tag="xp")
                nc.gpsimd.dma_start(xp[0:BC, :], xproj[BC * t:BC * (t + 1), 0:G])
                tb_ = L - 1 - t
                nc.gpsimd.dma_start(xp[32:48, :], xproj[BC * tb_:BC * (tb_ + 1), G:2 * G])

                for d in range(2):
                    ro = 32 * d
                    gt = gps.tile([BC, G], F32, tag=f"g{d}")
                    idd = i48[ro:ro + 16, ro:ro + 16]
                    for nh in range(2):
                        nc.tensor.matmul(gt[:, 512 * nh:512 * (nh + 1)], idd,
                                         xp[ro:ro + 16, 512 * nh:512 * (nh + 1)],
                                         start=True, stop=False,
                                         skip_group_check=True,
                                         tile_position=(ro, 0))
                        for kp in range(2):
                            lhsT = hT_all[:, base + 32 * kp + 16 * d:
                                          base + 32 * kp + 16 * d + 16]
                            nc.tensor.matmul(gt[:, 512 * nh:512 * (nh + 1)], lhsT,
                                             whhT_t[d][kp][:, 512 * nh:512 * (nh + 1)],
                                             start=False, stop=(kp == 1),
                                             skip_group_check=True)
                    # gate g-rows pre-scaled x2 on host: tanh(g) = 2*sig(2g)-1
                    sg = sgp.tile([BC, G], F32, tag=f"sg{d}")
                    nc.scalar.activation(sg[:], gt[:], AF.Sigmoid)

                    t1 = tmp.tile([BC, H], F32, tag=f"t1{d}")
                    nc.vector.tensor_tensor(out=t1[:], in0=sg[:, 256:512],
                                            in1=c_cur[d][:], op=ALU.mult)
                    t2 = tmp.tile([BC, H], F32, tag=f"t2{d}")
                    nc.vector.tensor_tensor(out=t2[:], in0=sg[:, 0:256],
                                            in1=sg[:, 768:1024], op=ALU.mult)
                    t3 = tmp.tile([BC, H], F32, tag=f"t3{d}")
                    nc.vector.scalar_tensor_tensor(out=t3[:], in0=t2[:], scalar=2.0,
                                                   in1=t1[:], op0=ALU.mult,
                                                   op1=ALU.add)
                    c_new = cvp.tile([BC, H], F32, tag=f"c{d}")
                    nc.vector.tensor_tensor(out=c_new[:], in0=t3[:],
                                            in1=sg[:, 0:256], op=ALU.subtract)
                    # h = sig(o)*tanh(c) = 2*sig(o)*sig(2c) - sig(o)
                    th = tmp.tile([BC, H], F32, tag=f"th{d}")
                    nc.scalar.activation(th[:], c_new[:], AF.Sigmoid, scale=2.0)
                    t4 = tmp.tile([BC, H], F32, tag=f"t4{d}")
                    nc.vector.tensor_tensor(out=t4[:], in0=sg[:, 512:768],
                                            in1=th[:], op=ALU.mult)
                    hnew = tmp.tile([BC, H], F32, tag=f"h{d}")
                    nc.vector.scalar_tensor_tensor(out=hnew[:], in0=t4[:], scalar=2.0,
                                                   in1=sg[:, 512:768], op0=ALU.mult,
                                                   op1=ALU.subtract)
                    for half in range(2):
                        tp = tps.tile([128, 16], F32, tag="tpB")
                        nc.tensor.transpose(tp[:], hnew[:, 128 * half:128 * (half + 1)],
                                            ident[:16, :16])
                        nc.vector.tensor_copy(
                            hT_all[:, nbase + 32 * half + 16 * d:
                                   nbase + 32 * half + 16 * d + 16], tp[:])
                    c_cur[d] = c_new

            # ---- Phase B': bulk output projection -> emissions ----
            woutT_t = [[None, None], [None, None]]
            for d in range(2):
                for kp in range(2):
                    wt = wb.tile([128, T], F32R, name=f"woutT{d}{kp}")
                    nc.gpsimd.dma_start(wt[:], woutT_d[d, kp])
                    woutT_t[d][kp] = wt

            psE = bctx.enter_context(tc.tile_pool(name="psE", bufs=2, space="PSUM"))
            for d in range(2):
                for j in range(NCH):
                    pse = psE.tile([128, T], F32, tag="pse")
                    s0 = 64 * (8 * j + 1)
                    blk = hT_all[:, s0:s0 + 512].rearrange("p (t u) -> p t u", u=64)
                    for kp in range(2):
                        a0 = 32 * kp + 16 * d
                        bstg = tmp.tile([128, 128], F32R, tag="bstg")
                        nc.vector.tensor_copy(bstg[:], blk[:, :, a0:a0 + 16])
                        nc.tensor.matmul(pse[:], bstg[:], woutT_t[d][kp],
                                         start=(kp == 0), stop=(kp == 1))
                    stg = tmp.tile([128, T], F32, tag="stg")
                    nc.vector.tensor_copy(stg[:], pse[:])
                    dst = em_f if d == 0 else em_b
                    for ti in range(8):
                        sx = 8 * j + ti
                        pos = sx if d == 0 else L - 1 - sx
                        nc.sync.dma_start(dst[BC * pos:BC * (pos + 1), :],
                                          stg[16 * ti:16 * (ti + 1), :])

        # ================= Phase C: two-sided CRF =================
        with ExitStack() as cctx:
            wc = cctx.enter_context(tc.tile_pool(name="wc", bufs=1))
            cps = cctx.enter_context(tc.tile_pool(name="cps", bufs=2, space="PSUM"))
            qps = cctx.enter_context(tc.tile_pool(name="qps", bufs=2, space="PSUM"))
            ctm = cctx.enter_context(tc.tile_pool(name="ctm", bufs=6))

            trs = wc.tile([T, T], F32)
            nc.sync.dma_start(trs[:], trans_d[:])
            bout74_sb = wc.tile([T, T], F32)
            nc.sync.dma_start(bout74_sb[:], bout_d[:])
            bout16_sb = wc.tile([BC, T], F32)
            nc.sync.dma_start(bout16_sb[:], bout16_d[:])
            start_sb = wc.tile([BC, T], F32)
            nc.sync.dma_start(start_sb[:], start_d[:])
            end_sb = wc.tile([BC, T], F32)
            nc.sync.dma_start(end_sb[:], end_d[:])
            nstat_sb = wc.tile([BC, 1], F32)
            nc.sync.dma_start(nstat_sb[:], nstat_d[:])

            trb = wc.tile([T, T], F32)
            nc.vector.tensor_tensor(out=trb[:], in0=trs[:], in1=bout74_sb[:],
                                    op=ALU.add)
            ET = wc.tile([T, T], F32)
            nc.scalar.activation(ET[:], trb[:], AF.Exp)
            etp = cps.tile([T, T], F32, tag="etp")
            nc.tensor.transpose(etp[:], ET[:], ident[:T, :T])
            ETT = wc.tile([T, T], F32)
            nc.vector.tensor_copy(ETT[:], etp[:])

            sv = wc.tile([BC, T], F32)
            nc.vector.tensor_tensor(out=sv[:], in0=start_sb[:], in1=bout16_sb[:],
                                    op=ALU.add)
            iota_f = wc.tile([BC, T], F32)
            nc.sync.dma_start(iota_f[:], iota_d[:])

            alpha = wc.tile([BC, T], F32)
            beta = wc.tile([BC, T], F32)
            acc_a = wc.tile([BC, T], F32)
            acc_b = wc.tile([BC, T], F32)

            def load_em(t, tagp):
                ef = ctm.tile([BC, T], F32, tag=tagp + "f")
                nc.sync.dma_start(ef[:], em_f[BC * t:BC * (t + 1), :])
                eb = ctm.tile([BC, T], F32, tag=tagp + "b")
                nc.sync.dma_start(eb[:], em_b[BC * t:BC * (t + 1), :])
                es = ctm.tile([BC, T], F32, tag=tagp + "s")
                nc.vector.tensor_tensor(out=es[:], in0=ef[:], in1=eb[:], op=ALU.add)
                return es

            ema0 = load_em(0, "e0")
            nc.vector.tensor_tensor(out=alpha[:], in0=ema0[:],
                                    in1=sv[:], op=ALU.add)
            eq0 = ctm.tile([BC, T], F32, tag="eq")
            nc.vector.tensor_scalar(out=eq0[:], in0=iota_f[:],
                                    scalar1=tags_sb[:, 0:1], scalar2=None, op0=ALU.is_equal)
            nc.vector.tensor_tensor(out=acc_a[:], in0=eq0[:], in1=ema0[:], op=ALU.mult)
            nc.vector.tensor_copy(beta[:], end_sb[:])
            nc.gpsimd.memset(acc_b[:], 0.0)

            def crf_step(k):
                do_a = k <= KMID
                ta, tb2 = k, L - k
                pt = ctm.tile([48, T], F32, tag="pt")
                if do_a:
                    ema = load_em(ta, "ea")
                    nm = ctm.tile([BC, 1], F32, tag="nm")
                    nc.vector.tensor_reduce(nm[:], alpha[:], axis=AX.X, op=ALU.max,
                                            negate=True)
                    nc.scalar.activation(pt[:BC, :], alpha[:], AF.Exp, bias=nm[:, :1])
                emb2 = load_em(tb2, "eb")
                ub = ctm.tile([BC, T], F32, tag="ub")
                nc.vector.tensor_tensor(out=ub[:], in0=emb2[:], in1=beta[:], op=ALU.add)
                nmb = ctm.tile([BC, 1], F32, tag="nmb")
                nc.vector.tensor_reduce(nmb[:], ub[:], axis=AX.X, op=ALU.max,
                                        negate=True)
                nc.scalar.activation(pt[32:48, :], ub[:], AF.Exp, bias=nmb[:, :1])

                ptp = cps.tile([T, 48], F32, tag="ptp")
                nc.tensor.transpose(ptp[:], pt[:], ident[:48, :48])
                pts = ctm.tile([T, 48], F32, tag="pts")
                nc.vector.tensor_copy(pts[:], ptp[:])

                if do_a:
                    qa = qps.tile([BC, T], F32, tag="qa")
                    nc.tensor.matmul(qa[:], pts[:, 0:BC], ET[:], start=True, stop=True)
                    la = ctm.tile([BC, T], F32, tag="la")
                    nc.scalar.activation(la[:], qa[:], AF.Ln)
                    nxa = ctm.tile([BC, T], F32, tag="nxa")
                    nc.vector.scalar_tensor_tensor(out=nxa[:], in0=la[:],
                                                   scalar=nm[:, :1], in1=ema[:],
                                                   op0=ALU.subtract, op1=ALU.add)
                    nc.vector.copy_predicated(
                        out=alpha[:], mask=maskv_sb[:, ta:ta + 1].to_broadcast([BC, T]),
                        data=nxa[:])
                    eqa = ctm.tile([BC, T], F32, tag="eq")
                    nc.vector.tensor_scalar(out=eqa[:],
                                            in0=iota_f[:],
                                            scalar1=tags_sb[:, ta:ta + 1],
                                            scalar2=None, op0=ALU.is_equal)
                    ga = ctm.tile([BC, T], F32, tag="ga")
                    nc.vector.tensor_tensor(out=ga[:], in0=eqa[:], in1=ema[:],
                                            op=ALU.mult)
                    nc.vector.tensor_tensor(out=acc_a[:], in0=acc_a[:], in1=ga[:],
                                            op=ALU.add)

                qb = qps.tile([BC, T], F32, tag="qb")
                nc.tensor.matmul(qb[:], pts[:, 32:48], ETT[:], start=True, stop=True)
                lb = ctm.tile([BC, T], F32, tag="lb")
                nc.scalar.activation(lb[:], qb[:], AF.Ln)
                nxb = ctm.tile([BC, T], F32, tag="nxb")
                nc.vector.tensor_scalar(out=nxb[:], in0=lb[:], scalar1=nmb[:, :1],
                                        scalar2=None, op0=ALU.subtract)
                nc.vector.copy_predicated(
                    out=beta[:], mask=maskv_sb[:, tb2:tb2 + 1].to_broadcast([BC, T]),
                    data=nxb[:])
                eqb = ctm.tile([BC, T], F32, tag="eq")
                nc.vector.tensor_scalar(out=eqb[:],
                                        in0=iota_f[:],
                                        scalar1=tags_sb[:, tb2:tb2 + 1],
                                        scalar2=None, op0=ALU.is_equal)
                gb = ctm.tile([BC, T], F32, tag="gb")
                nc.vector.tensor_tensor(out=gb[:], in0=eqb[:], in1=emb2[:], op=ALU.mult)
                nc.vector.tensor_tensor(out=acc_b[:], in0=acc_b[:], in1=gb[:],
                                        op=ALU.add)

            for k in range(1, L - KMID):
                crf_step(k)

            # ---- finalize: den = lse(alpha + beta), llh = nstat + gold - den ----
            uf = ctm.tile([BC, T], F32, tag="uf")
            nc.vector.tensor_tensor(out=uf[:], in0=alpha[:], in1=beta[:], op=ALU.add)
            nmf = wc.tile([BC, 1], F32)
            nc.vector.tensor_reduce(nmf[:], uf[:], axis=AX.X, op=ALU.max, negate=True)
            pf = ctm.tile([BC, T], F32, tag="pf")
            sf = wc.tile([BC, 1], F32)
            nc.scalar.activation(pf[:], uf[:], AF.Exp, bias=nmf[:, :1],
                                 accum_out=sf[:, :1])
            lnf = wc.tile([BC, 1], F32)
            nc.scalar.activation(lnf[:], sf[:], AF.Ln)
            den = wc.tile([BC, 1], F32)
            nc.vector.tensor_tensor(out=den[:], in0=lnf[:], in1=nmf[:], op=ALU.subtract)
            gsa = wc.tile([BC, 1], F32)
            nc.vector.tensor_reduce(gsa[:], acc_a[:], axis=AX.X, op=ALU.add)
            gsb = wc.tile([BC, 1], F32)
            nc.vector.tensor_reduce(gsb[:], acc_b[:], axis=AX.X, op=ALU.add)
            r1 = wc.tile([BC, 1], F32)
            nc.vector.tensor_tensor(out=r1[:], in0=gsa[:], in1=gsb[:], op=ALU.add)
            r2 = wc.tile([BC, 1], F32)
            nc.vector.tensor_tensor(out=r2[:], in0=r1[:], in1=nstat_sb[:], op=ALU.add)
            llh_t = wc.tile([BC, 1], F32)
            nc.vector.tensor_tensor(out=llh_t[:], in0=r2[:], in1=den[:], op=ALU.subtract)
            nc.sync.dma_start(llh_d[:], llh_t[:])

    nc.compile()
    return nc


def _injb_const():
    m = np.zeros((48, 32), np.float32)
    m[32:48, 0:16] = np.eye(16, dtype=np.float32)
    return m


def _host_prep(inputs):
    """Pure index/permutation transforms of the inputs -> per-core input maps."""
    f32 = np.float32
    ii = {k: np.asarray(v) for k, v in inputs.items()}
    input_ids = ii["input_ids"].astype(np.int32)
    tag_ids = ii["tag_ids"].astype(np.int64)
    lengths = np.maximum(ii["lengths"].astype(np.int64), 1)
    embed = np.ascontiguousarray(ii["embed"], dtype=f32)
    trans = np.asarray(ii["trans"], f32)
    start_t = np.asarray(ii["start_t"], f32)
    end_t = np.asarray(ii["end_t"], f32)
    bout = np.asarray(ii["bout"], f32)

    # gate reorder: torch [i, f, g, o] -> [i, f, o, g]
    perm = np.concatenate([np.arange(0, 256), np.arange(256, 512),
                           np.arange(768, 1024), np.arange(512, 768)])
    Wih_f = np.asarray(ii["Wih_f"], f32)[perm]
    Wih_b = np.asarray(ii["Wih_b"], f32)[perm]
    Whh_f = np.asarray(ii["Whh_f"], f32)[perm]
    Whh_b = np.asarray(ii["Whh_b"], f32)[perm]
    b_f = (np.asarray(ii["bih_f"], f32) + np.asarray(ii["bhh_f"], f32))[perm]
    b_b = (np.asarray(ii["bih_b"], f32) + np.asarray(ii["bhh_b"], f32))[perm]
    Wout = np.asarray(ii["Wout"], f32)

    gsc = np.ones((1024, 1), f32)
    gsc[768:1024] = 2.0
    Wih_f = Wih_f * gsc
    Wih_b = Wih_b * gsc
    Whh_f = Whh_f * gsc
    Whh_b = Whh_b * gsc
    b_f = b_f * gsc[:, 0]
    b_b = b_b * gsc[:, 0]
    W_all = np.concatenate([Wih_f, Wih_b], axis=0)          # [2G, E]
    wallT = np.ascontiguousarray(W_all.T.reshape(4, 128, 2 * G))
    bias_all = np.concatenate([b_f, b_b])[None, :]          # [1, 2G]
    whhT = np.stack([np.ascontiguousarray(Whh_f.T.reshape(2, 128, G)),
                     np.ascontiguousarray(Whh_b.T.reshape(2, 128, G))])
    woutT = np.stack([np.ascontiguousarray(Wout[:, :H].T.reshape(2, 128, T)),
                      np.ascontiguousarray(Wout[:, H:].T.reshape(2, 128, T))])
    h0 = np.asarray(ii["h0"], f32)
    c0 = np.asarray(ii["c0"], f32)

    shared = dict(embed=embed, wallT=wallT,
                  bias_all=np.ascontiguousarray(np.broadcast_to(bias_all, (128, 2 * G))),
                  whhT=whhT, woutT=woutT, transm=trans,
                  bout74=np.ascontiguousarray(np.broadcast_to(bout[None, :], (T, T))),
                  bout16=np.ascontiguousarray(np.broadcast_to(bout[None, :], (BC, T))),
                  start16=np.ascontiguousarray(np.broadcast_to(start_t[None, :], (BC, T))),
                  end16=np.ascontiguousarray(np.broadcast_to(end_t[None, :], (BC, T))),
                  iota16=np.ascontiguousarray(np.broadcast_to(
                      np.arange(T, dtype=f32)[None, :], (BC, T))),
                  injb=_injb_const())

    tpos = np.arange(L)
    in_maps = []
    for c in range(NC):
        sl = slice(c * BC, (c + 1) * BC)
        ids_lin = input_ids[sl].T.reshape(-1)               # t-major [R]
        ids_arr = np.ascontiguousarray(ids_lin.reshape(R // 128, 128).T).astype(np.int32)
        tg = tag_ids[sl]                                    # [BC, L]
        ln = lengths[sl]                                    # [BC]
        valid = tpos[None, :] < ln[:, None]                 # [BC, L]
        tags_eff = np.where(valid, tg, 999).astype(f32)
        maskv = valid.astype(np.int32)
        # static numerator terms (start/trans/end/bout lookups)
        nstat = start_t[tg[:, 0]].astype(np.float64)
        m = valid[:, 1:]
        nstat += (trans[tg[:, :-1], tg[:, 1:]].astype(np.float64) * m).sum(axis=1)
        nstat += end_t[tg[np.arange(BC), ln - 1]]
        nstat += (bout[tg].astype(np.float64) * valid).sum(axis=1)
        im = dict(shared)
        im.update(ids=ids_arr, tagsf=np.ascontiguousarray(tags_eff),
                  maskv=np.ascontiguousarray(maskv),
                  nstat=nstat.astype(f32)[:, None],
                  h0c=np.ascontiguousarray(h0[:, sl].reshape(2 * BC, H)),
                  c0c=np.ascontiguousarray(c0[:, sl].reshape(2 * BC, H)))
        in_maps.append(im)
    return in_maps


def kernel(**inputs):
    from concourse.bass_utils import run_bass_kernel_spmd

    if "nc" not in _CACHE:
        _CACHE["nc"] = _build()
    nc = _CACHE["nc"]
    in_maps = _host_prep(inputs)
    trace = bool(int(os.environ.get("KERNEL_TRACE", "0")))
    try:
        res = run_bass_kernel_spmd(nc, in_maps, core_ids=list(range(NC)), trace=trace)
        if trace:
            _CACHE["exec_time_ns"] = res.exec_time_ns
    except (ImportError, ModuleNotFoundError):
        res = run_bass_kernel_spmd(nc, in_maps, core_ids=list(range(NC)), trace=False)
    llh = np.concatenate([r["llh"][:, 0] for r in res.results])
    return np.asarray(-llh.mean(), dtype=np.float32)
